# revision 1
# baseline (speedup 1.0000x reference)
"""DCRNN (2-layer encoder/decoder DCGRU, N=512 nodes, B=32, U=64, K=2, 2 supports)
Trainium2 Bass/Tile kernel, data-parallel over batch across 8 NeuronCores.

Key reformulation: the Chebyshev graph conv
    gconv(X) = sum_m T_m(S) @ X @ W_m,  T in {I, S_A, 2S_A^2-I, S_B, 2S_B^2-I}
with the T_m precomputed on host. Per gconv:
  stage 1 (dense):     A_m = X @ W_m      -- activations kept channel-on-partition
                       (X^T used as matmul lhsT, W as rhs -> A node-on-partition)
  stage 2 (diffusion): out^T = sum_m (T_m A_m)^T   -- A_m as lhsT, T_m^T as rhs,
                       accumulated over m in PSUM; bias+sigmoid/tanh fused into
                       the PSUM->SBUF move.
This needs no tensor transposes at runtime. All weights/supports/state resident
in SBUF for the whole 24-step recurrence; only encoder inputs in / decoder
outputs out touch HBM.
"""

import sys

sys.path.insert(0, "/opt/trn_rl_repo")

import numpy as np

import concourse.bass as bass
import concourse.mybir as mybir
import concourse.tile as tile
from concourse import bacc, bass_utils

# Model dims (fixed by the problem)
N = 512
T_ENC = 12
HOR = 12
U = 64
NM = 5  # num diffusion matrices (I + 2 per support * 2 supports)
B = 32
NCORES = 8
BL = B // NCORES  # local batch = 4
BI = BL * N  # 2048: the (b, node) free dim
C0 = 1 + U  # 65 input channels, layer 0
C1 = U + U  # 128 input channels, layer 1
KCH = N // 128  # 4 node chunks

F32 = mybir.dt.float32
F32R = mybir.dt.float32r
BF16 = mybir.dt.bfloat16
AF = mybir.ActivationFunctionType

S2_BF16 = False  # stage-2 (diffusion) matmuls in bf16
LDW_OPT = True  # enable walrus ldw-opt pass (default cmdline disables it)

_ldw_patched = False


def _patch_ldw_opt():
    global _ldw_patched
    if _ldw_patched or not LDW_OPT:
        return
    _ldw_patched = True
    orig = bass_utils.bir_verify_and_optimise

    def patched(tmpdir, inp="bir.json", outp="file.neff", arch=None, *, dve_root=None):
        import concourse.bass_utils as bu

        real_run = bu.run_command

        def run_hook(cmd, **kw):
            cmd = [
                c.replace("--enable-ldw-opt=false", "--enable-ldw-opt=true")
                for c in cmd
            ]
            return real_run(cmd, **kw)

        bu.run_command = run_hook
        try:
            return orig(tmpdir, inp, outp, arch, dve_root=dve_root)
        finally:
            bu.run_command = real_run

    bass_utils.bir_verify_and_optimise = patched


def _r(ap):
    """View an fp32 AP as float32r for full-rate PE matmuls."""
    return ap.bitcast(F32R)


def _build_program(n_enc=T_ENC, n_dec=HOR, mm_dtype="f32r"):
    _patch_ldw_opt()
    nc = bacc.Bacc("TRN2", target_bir_lowering=False, debug=False)

    rr = _r if mm_dtype == "f32r" else (lambda ap: ap)

    # ---- DRAM I/O ----
    d_xenc = nc.dram_tensor("xenc", [n_enc, BI], F32, kind="ExternalInput")
    d_tm = nc.dram_tensor(
        "tmats", [NM * KCH * 128, 512], BF16 if S2_BF16 else F32, kind="ExternalInput"
    )
    d_w = {}
    for pfx in ("e", "d"):
        for lyr, c_in in ((0, C0), (1, C1)):
            d_w[f"{pfx}wg{lyr}"] = nc.dram_tensor(
                f"{pfx}wg{lyr}", [c_in, NM * 2 * U], F32, kind="ExternalInput"
            )
            d_w[f"{pfx}wc{lyr}"] = nc.dram_tensor(
                f"{pfx}wc{lyr}", [c_in, NM * U], F32, kind="ExternalInput"
            )
            d_w[f"{pfx}bgr{lyr}"] = nc.dram_tensor(
                f"{pfx}bgr{lyr}", [U, 1], F32, kind="ExternalInput"
            )
            d_w[f"{pfx}bgu{lyr}"] = nc.dram_tensor(
                f"{pfx}bgu{lyr}", [U, 1], F32, kind="ExternalInput"
            )
            d_w[f"{pfx}bgn{lyr}"] = nc.dram_tensor(
                f"{pfx}bgn{lyr}", [U, 1], F32, kind="ExternalInput"
            )
            d_w[f"{pfx}bc{lyr}"] = nc.dram_tensor(
                f"{pfx}bc{lyr}", [U, 1], F32, kind="ExternalInput"
            )
    d_pw = nc.dram_tensor("pw", [U, 2], F32, kind="ExternalInput")
    d_zeros = nc.dram_tensor("zeros", [U, BI], F32, kind="ExternalInput")
    d_pb = nc.dram_tensor("pb", [1, 1], F32, kind="ExternalInput")
    d_out = nc.dram_tensor("outs", [n_dec, BI], F32, kind="ExternalOutput")

    with tile.TileContext(nc) as tc:
        _body(tc, n_enc, n_dec, rr, d_xenc, d_tm, d_w, d_pw, d_pb, d_zeros, d_out)
    nc.compile()
    return nc


def _body(tc, n_enc, n_dec, rr, d_xenc, d_tm, d_w, d_pw, d_pb, d_zeros, d_out):
    nc = tc.nc
    consts = tc.alloc_tile_pool(name="consts", bufs=1)
    work = tc.alloc_tile_pool(name="work", bufs=1)
    gpool = tc.alloc_tile_pool(name="gpool", bufs=2)
    ag_pool = tc.alloc_tile_pool(name="agp", bufs=10)
    ac_pool = tc.alloc_tile_pool(name="acp", bufs=6)
    ps1 = tc.alloc_tile_pool(name="ps1", bufs=3, space="PSUM")
    ps2 = tc.alloc_tile_pool(name="ps2", bufs=4, space="PSUM")

    # ---- resident constants ----
    s2dt = BF16 if S2_BF16 else F32
    s2r = (lambda ap: ap) if S2_BF16 else _r
    tm_sb = consts.tile([128, NM, KCH, 512], s2dt, name="tm_sb")
    for m in range(NM):
        for k in range(KCH):
            row = (m * KCH + k) * 128
            nc.sync.dma_start(
                out=s2r(tm_sb[:, m, k, :]), in_=s2r(d_tm[row : row + 128, :])
            )

    w_sb = {}
    for key, dt in d_w.items():
        shape = list(dt.shape)
        w_sb[key] = consts.tile(shape, F32, name=f"sb_{key}")
        if key[1] == "w":
            nc.sync.dma_start(out=_r(w_sb[key][:, :]), in_=_r(dt[:, :]))
        else:
            nc.sync.dma_start(out=w_sb[key], in_=dt[:, :])
    pw_sb = consts.tile([U, 2], F32, name="pw_sb")
    nc.sync.dma_start(out=_r(pw_sb[:, :]), in_=_r(d_pw[:, :]))
    pb_sb = consts.tile([1, 1], F32, name="pb_sb")
    nc.sync.dma_start(out=pb_sb, in_=d_pb[:, :])

    # ---- persistent state (channel-on-partition, free dim = (b, node)) ----
    X0 = work.tile([C0, BI], F32, name="X0")  # [h0 ; x]
    X0c = work.tile([C0, BI], F32, name="X0c")  # [r*h0 ; x]
    X1 = work.tile([C1, BI], F32, name="X1")  # [h0 ; h1]
    X1c = work.tile([C1, BI], F32, name="X1c")  # [h0 ; r*h1]
    h1t = work.tile([U, BI], F32, name="h1t")  # h1 at partition base 0

    xstage = work.tile([1, BI], F32, name="xstage")

    nc.sync.dma_start(out=_r(X0[0:U, :]), in_=_r(d_zeros[:, :]))
    nc.sync.dma_start(out=_r(X1[U:C1, :]), in_=_r(d_zeros[:, :]))
    nc.sync.dma_start(out=_r(h1t[:, :]), in_=_r(d_zeros[:, :]))

    # collapse all load/init dependencies into one semaphore so the first
    # consumers don't exceed per-instruction sync-wait slots
    tc.strict_bb_all_engine_barrier()

    def cell_phases(X, Xc, c_in, rh_lo, h_src, wg, bgr, bgu, bgn, wc, bc,
                    h_writer, post):
        """One DCGRU cell, split into per-batch-pair phases so independent
        pairs' matmuls cover each other's activation/elementwise tails.
        Returns (gate_phase, cand_phase), each callable with p in {0, 1}."""
        R = gpool.tile([U, BI], F32, tag="R", name="R", bufs=1)
        Uu = gpool.tile([U, BI], F32, tag="Uu", name="Uu", bufs=1)
        Wu = gpool.tile([U, BI], F32, tag="Wu", name="Wu", bufs=1)
        uh = gpool.tile([U, BI], F32, tag="uh", name="uh", bufs=1)
        Ct = gpool.tile([U, BI], F32, tag="Ct", name="Ct", bufs=1)
        wc_t = gpool.tile([U, BI], F32, tag="wct", name="wc_t", bufs=1)
        ag = {}
        ac = {}

        def gate_phase(p):
            # stage 1: A_m = X @ Wg_m for m=1..4 (m=0 folded into stage 2)
            for b in (2 * p, 2 * p + 1):
                for k in range(KCH):
                    pg = ps1.tile([128, 512], F32, tag="s1", name="pg")
                    lhsT = X[0:c_in, b * N + k * 128 : b * N + (k + 1) * 128]
                    nc.tensor.matmul(
                        pg, rr(lhsT), rr(wg[:, 128:640]), start=True, stop=True
                    )
                    a = ag_pool.tile([128, 4 * 128], s2dt, tag="ag", name="ag")
                    ag[(b, k)] = a
                    nc.scalar.copy(out=s2r(a[:, :]), in_=pg)
            # stage 2: acc = X @ Wg_0 + sum_{m>0} (T_m A_m)^T, fused sigmoid
            for b in (2 * p, 2 * p + 1):
                acc = ps2.tile([128, 512], F32, tag="s2", name="accg")
                nc.tensor.matmul(
                    acc,
                    rr(wg[:, 0:128]),
                    rr(X[0:c_in, b * N : (b + 1) * N]),
                    start=True,
                    stop=False,
                )
                for m in range(1, NM):
                    for k in range(KCH):
                        nc.tensor.matmul(
                            acc,
                            s2r(ag[(b, k)][:, (m - 1) * 128 : m * 128]),
                            s2r(tm_sb[:, m, k, :]),
                            start=False,
                            stop=(m == NM - 1 and k == KCH - 1),
                        )
                bcols = slice(b * N, (b + 1) * N)
                nc.scalar.activation(
                    out=R[:, bcols], in_=acc[0:U, :], func=AF.Sigmoid,
                    bias=bgr[:, 0:1], scale=1.0,
                )
                nc.scalar.activation(
                    out=Uu[:, bcols], in_=acc[U : 2 * U, :], func=AF.Sigmoid,
                    bias=bgu[:, 0:1], scale=1.0,
                )
                nc.scalar.activation(
                    out=Wu[:, bcols],
                    in_=acc[U : 2 * U, :],
                    func=AF.Sigmoid,
                    bias=bgn[:, 0:1],
                    scale=-1.0,
                )
            pcols = slice(2 * p * N, 2 * (p + 1) * N)
            # r*h -> candidate input rows; u*h for the GRU blend
            nc.vector.tensor_mul(
                out=_r(Xc[rh_lo : rh_lo + U, pcols]),
                in0=R[:, pcols],
                in1=h_src[:, pcols],
            )
            nc.vector.tensor_mul(
                out=uh[:, pcols],
                in0=Uu[:, pcols],
                in1=h_src[:, pcols],
            )

        def cand_phase(p):
            for b in (2 * p, 2 * p + 1):
                for k in range(KCH):
                    pc = ps1.tile([128, 512], F32, tag="s1", name="pc")
                    lhsT = Xc[0:c_in, b * N + k * 128 : b * N + (k + 1) * 128]
                    nc.tensor.matmul(
                        pc[:, 0:320], rr(lhsT), rr(wc[:, :]), start=True, stop=True
                    )
                    if b % 2 == 0:
                        a = ac_pool.tile([128, NM, 2, U], s2dt, tag="ac", name="ac")
                        ac[(p, k)] = a
                    dst = s2r(ac[(p, k)][:, :, b % 2, :])
                    src_v = pc[:, 0:320].rearrange("p (m u) -> p m u", m=NM)
                    nc.vector.tensor_copy(out=dst, in_=src_v)
            acc = ps2.tile([128, 512], F32, tag="s2", name="accc")
            for m in range(NM):
                for k in range(KCH):
                    nc.tensor.matmul(
                        acc,
                        s2r(ac[(p, k)][:, m, :, :]),
                        s2r(tm_sb[:, m, k, :]),
                        start=(m == 0 and k == 0),
                        stop=(m == NM - 1 and k == KCH - 1),
                    )
            for half in range(2):
                b = 2 * p + half
                bcols = slice(b * N, (b + 1) * N)
                nc.scalar.activation(
                    out=Ct[:, bcols],
                    in_=acc[half * U : (half + 1) * U, :],
                    func=AF.Tanh,
                    bias=bc[:, 0:1],
                    scale=1.0,
                )
            pcols = slice(2 * p * N, 2 * (p + 1) * N)
            # h_new = u*h + (1-u)*c
            nc.vector.tensor_mul(out=wc_t[:, pcols], in0=Wu[:, pcols], in1=Ct[:, pcols])
            nc.vector.tensor_add(
                out=_r(h_writer(p, pcols)), in0=uh[:, pcols], in1=wc_t[:, pcols]
            )
            post(p, pcols)

        return gate_phase, cand_phase

    def l0_writer(p, pcols):
        return X0[0:U, pcols]

    def l0_post(p, pcols):
        # propagate h0 into the layer-1 input tiles (X1c rows are [r*h1 ; h0])
        nc.gpsimd.tensor_copy(out=_r(X1[0:U, pcols]), in_=_r(X0[0:U, pcols]))
        nc.gpsimd.tensor_copy(out=_r(X1c[U:C1, pcols]), in_=_r(X0[0:U, pcols]))

    def l1_writer(p, pcols):
        return h1t[:, pcols]

    def l1_post(p, pcols):
        nc.gpsimd.tensor_copy(out=_r(X1[U:C1, pcols]), in_=_r(h1t[:, pcols]))

    def proj_phase(p):
        # projection for pair p: out = h1 . pw + pb -> feeds back as x row
        for q in (2 * p, 2 * p + 1):
            pp = ps2.tile([2, 512], F32, tag="s2", name="pp")
            nc.tensor.matmul(
                pp,
                rr(pw_sb[:, 0:2]),
                rr(h1t[:, q * 512 : (q + 1) * 512]),
                start=True,
                stop=True,
            )
            nc.scalar.activation(
                out=_r(X0[U:C0, q * 512 : (q + 1) * 512]),
                in_=pp[0:1, :],
                func=AF.Identity,
                bias=pb_sb[:, 0:1],
                scale=1.0,
            )
        pcols = slice(2 * p * N, 2 * (p + 1) * N)
        nc.vector.tensor_copy(out=_r(X0c[U:C0, pcols]), in_=_r(X0[U:C0, pcols]))

    def run_step(pfx, dec_t=None):
        g0, c0 = cell_phases(
            X0, X0c, C0, 0, X0[0:U, :],
            w_sb[f"{pfx}wg0"], w_sb[f"{pfx}bgr0"], w_sb[f"{pfx}bgu0"],
            w_sb[f"{pfx}bgn0"], w_sb[f"{pfx}wc0"], w_sb[f"{pfx}bc0"],
            l0_writer, l0_post,
        )
        g1, c1 = cell_phases(
            X1, X1c, C1, 0, h1t[:, :],
            w_sb[f"{pfx}wg1"], w_sb[f"{pfx}bgr1"], w_sb[f"{pfx}bgu1"],
            w_sb[f"{pfx}bgn1"], w_sb[f"{pfx}wc1"], w_sb[f"{pfx}bc1"],
            l1_writer, l1_post,
        )
        g0(0); g0(1); c0(0); c0(1)
        g1(0); g1(1); c1(0)
        if dec_t is None:
            c1(1)
        else:
            proj_a_after = c1  # readability: proj(p) right after c1(p)
            proj_phase(0)
            c1(1)
            proj_phase(1)
            nc.sync.dma_start(out=d_out[dec_t : dec_t + 1, :], in_=X0[U:C0, :])

    # ================= encoder =================
    for t in range(n_enc):
        nc.sync.dma_start(out=xstage, in_=d_xenc[t : t + 1, :])
        nc.vector.tensor_copy(out=_r(X0[U:C0, :]), in_=xstage)
        nc.vector.tensor_copy(out=_r(X0c[U:C0, :]), in_=xstage)
        run_step("e")

    # ================= decoder =================
    nc.sync.dma_start(out=_r(X0[U:C0, :]), in_=_r(d_zeros[0:1, :]))
    nc.sync.dma_start(out=_r(X0c[U:C0, :]), in_=_r(d_zeros[0:1, :]))
    for t in range(n_dec):
        run_step("d", dec_t=t)

    for pool in (ps2, ps1, ac_pool, ag_pool, gpool, work, consts):
        pool.release()


# --------------------------------------------------------------------------
# host-side packing
# --------------------------------------------------------------------------
def _prep_shared(inputs):
    sup = np.asarray(inputs["supports"], np.float64)
    eye = np.eye(N, dtype=np.float64)
    tms = [
        eye,
        sup[0],
        2.0 * (sup[0] @ sup[0]) - eye,
        sup[1],
        2.0 * (sup[1] @ sup[1]) - eye,
    ]
    tmats = np.stack([t.T for t in tms]).astype(np.float32)  # [m, j, i]
    tmats = tmats.reshape(NM * KCH * 128, 512)

    if S2_BF16:
        import ml_dtypes

        tmats = tmats.astype(ml_dtypes.bfloat16)
    shared = {"tmats": np.ascontiguousarray(tmats)}
    for pfx, name in (("e", "enc"), ("d", "dec")):
        for lyr, c_in in ((0, C0), (1, C1)):
            wg = np.asarray(inputs[f"{name}{lyr}_Wg"], np.float32)
            wc = np.asarray(inputs[f"{name}{lyr}_Wc"], np.float32)
            wg = wg.reshape(c_in, NM * 2 * U)
            wc = wc.reshape(c_in, NM * U)
            bg = np.asarray(inputs[f"{name}{lyr}_bg"], np.float32)
            bc = np.asarray(inputs[f"{name}{lyr}_bc"], np.float32)
            wg_r = wg.reshape(c_in, NM, 2 * U).reshape(c_in, NM * 2 * U)
            wc_r = wc.reshape(c_in, NM, U).reshape(c_in, NM * U)
            if lyr == 0:
                perm = np.r_[1:c_in, 0]  # rows [h..., x]
                wg_r = wg_r[perm]
                wc_r = wc_r[perm]
            else:
                # X1c rows are [r*h1 ; h0]: candidate weight rows follow
                wc_r = wc_r[np.r_[U:c_in, 0:U]]
            shared[f"{pfx}wg{lyr}"] = np.ascontiguousarray(wg_r)
            shared[f"{pfx}wc{lyr}"] = np.ascontiguousarray(wc_r)
            shared[f"{pfx}bgr{lyr}"] = np.ascontiguousarray(bg[:U].reshape(U, 1))
            shared[f"{pfx}bgu{lyr}"] = np.ascontiguousarray(bg[U:].reshape(U, 1))
            shared[f"{pfx}bgn{lyr}"] = np.ascontiguousarray(-bg[U:].reshape(U, 1))
            shared[f"{pfx}bc{lyr}"] = np.ascontiguousarray(bc.reshape(U, 1))
    pw = np.asarray(inputs["proj_W"], np.float32).reshape(U, 1)
    shared["pw"] = np.ascontiguousarray(
        np.concatenate([pw, np.zeros((U, 1), np.float32)], axis=1)
    )
    shared["pb"] = np.asarray(inputs["proj_b"], np.float32).reshape(1, 1)
    shared["zeros"] = np.zeros((U, BI), np.float32)
    return shared


def _make_in_maps(inputs, n_enc=T_ENC):
    shared = _prep_shared(inputs)
    x = np.asarray(inputs["inputs"], np.float32)  # (T, B, N)
    in_maps = []
    for c in range(NCORES):
        m = dict(shared)
        m["xenc"] = np.ascontiguousarray(
            x[:n_enc, c * BL : (c + 1) * BL, :].reshape(n_enc, BI)
        )
        in_maps.append(m)
    return in_maps


_PROG_CACHE = {}


def _get_program(n_enc=T_ENC, n_dec=HOR):
    key = (n_enc, n_dec)
    if key not in _PROG_CACHE:
        _PROG_CACHE[key] = _build_program(n_enc, n_dec)
    return _PROG_CACHE[key]


def _run(inputs, n_enc=T_ENC, n_dec=HOR, **kw):
    nc = _get_program(n_enc, n_dec)
    in_maps = _make_in_maps(inputs, n_enc)
    res = bass_utils.run_bass_kernel_spmd(nc, in_maps, core_ids=list(range(NCORES)), **kw)
    out = np.empty((n_dec, B, N), np.float32)
    for c in range(NCORES):
        out[:, c * BL : (c + 1) * BL, :] = res.results[c]["outs"].reshape(n_dec, BL, N)
    return out.reshape(n_dec, B, N), res


def kernel(**inputs) -> np.ndarray:
    out, _ = _run(inputs)
    return out.reshape(HOR, B, N)



# revision 9
# speedup vs baseline: 1.3950x; 1.3950x over previous
"""DCRNN (2-layer encoder/decoder DCGRU, N=512 nodes, B=32, U=64, K=2, 2 supports)
Trainium2 Bass/Tile kernel, data-parallel over batch across 8 NeuronCores.

Formulation: gconv(X) = sum_m T_m @ X @ W_m with T_m precomputed on host
(m=0 is the identity and is folded into stage 2 as a direct X @ W_0 matmul).
  stage 1 (dense):     A_m = X @ W_m, m=1..4  (X-chunk as lhsT -> node-major A)
  stage 2 (diffusion): out = X @ W_0 + sum_m (T_m A_m)^T, accumulated in PSUM,
                       bias+sigmoid/tanh fused into the PSUM->SBUF activation.
All matmul operands bf16 (or fp8e4m3 with DoubleRow for the diffusion stage:
two 128-row node chunks contracted per matmul at 2x rate). State tiles bf16.
Layout avoids every partition-shift copy:
  X0  [65,BI]  rows 0:64 h0,    row 64 x      (L0 gate lhsT)
  X0c [65,BI]  rows 0:64 r0*h0, row 64 x      (L0 cand lhsT)
  X1  [128,BI] rows 0:64 h0',   rows 64:128 h1        (L1 gate lhsT)
  X1c [128,BI] rows 0:64 h0',   rows 64:128 r1*h1     (L1 cand lhsT)
h1 and r1*h1 are written at partition base 64 directly by DVE (cross-base ok).
"""

import sys

sys.path.insert(0, "/opt/trn_rl_repo")

import numpy as np

import concourse.bass as bass
import concourse.mybir as mybir
import concourse.tile as tile
from concourse import bacc, bass_utils

# Model dims (fixed by the problem)
N = 512
T_ENC = 12
HOR = 12
U = 64
NM = 5  # diffusion matrices (I + 2 per support * 2 supports)
B = 32
NCORES = 8
BL = B // NCORES  # local batch = 4
BI = BL * N  # 2048: the (b, node) free dim
C0 = 1 + U  # 65 input channels, layer 0
C1 = U + U  # 128 input channels, layer 1
KCH = N // 128  # 4 node chunks

F32 = mybir.dt.float32
BF16 = mybir.dt.bfloat16
FP8 = mybir.dt.float8e4
AF = mybir.ActivationFunctionType
DR = mybir.MatmulPerfMode.DoubleRow

# stage-2 diffusion dtype per path: "fp8" (DoubleRow) or "bf16"
import os as _os

S2_GATE = _os.environ.get("S2_GATE", "fp8")
S2_CAND = _os.environ.get("S2_CAND", "fp8")
# walrus ldw-opt pass chokes on DoubleRow Ldweights; only enable when no fp8 path
LDW_OPT = _os.environ.get("LDW_OPT", "0") == "1"

_ldw_patched = False


def _patch_ldw_opt():
    global _ldw_patched
    if _ldw_patched or not LDW_OPT:
        return
    _ldw_patched = True
    orig = bass_utils.bir_verify_and_optimise

    def patched(tmpdir, inp="bir.json", outp="file.neff", arch=None, *, dve_root=None):
        import concourse.bass_utils as bu

        real_run = bu.run_command

        def run_hook(cmd, **kw):
            cmd = [
                c.replace("--enable-ldw-opt=false", "--enable-ldw-opt=true")
                for c in cmd
            ]
            return real_run(cmd, **kw)

        bu.run_command = run_hook
        try:
            return orig(tmpdir, inp, outp, arch, dve_root=dve_root)
        finally:
            bu.run_command = real_run

    bass_utils.bir_verify_and_optimise = patched


def _build_program(n_enc=T_ENC, n_dec=HOR):
    _patch_ldw_opt()
    nc = bacc.Bacc("TRN2", target_bir_lowering=False, debug=False)

    # ---- DRAM I/O ----
    d_xenc = nc.dram_tensor("xenc", [n_enc, BI], BF16, kind="ExternalInput")
    d_tm = {}
    dts = set((S2_GATE, S2_CAND))
    for s2 in dts:
        dt_ = FP8 if s2 == "fp8" else BF16
        d_tm[s2] = nc.dram_tensor(
            f"tm_{s2}", [NM * KCH * 128, 512], dt_, kind="ExternalInput"
        )
    d_w = {}
    for pfx in ("e", "d"):
        for lyr, c_in in ((0, C0), (1, C1)):
            d_w[f"{pfx}wg{lyr}"] = nc.dram_tensor(
                f"{pfx}wg{lyr}", [c_in, NM * 2 * U], BF16, kind="ExternalInput"
            )
            d_w[f"{pfx}wc{lyr}"] = nc.dram_tensor(
                f"{pfx}wc{lyr}", [c_in, NM * U], BF16, kind="ExternalInput"
            )
            d_w[f"{pfx}bg{lyr}"] = nc.dram_tensor(
                f"{pfx}bg{lyr}", [2 * U, 1], F32, kind="ExternalInput"
            )
            d_w[f"{pfx}bc{lyr}"] = nc.dram_tensor(
                f"{pfx}bc{lyr}", [U, 1], F32, kind="ExternalInput"
            )
    d_pw = nc.dram_tensor("pw", [U, 2], BF16, kind="ExternalInput")
    d_pb = nc.dram_tensor("pb", [1, 1], F32, kind="ExternalInput")
    d_out = nc.dram_tensor("outs", [n_dec, BI], F32, kind="ExternalOutput")

    with tile.TileContext(nc) as tc:
        _body(tc, n_enc, n_dec, d_xenc, d_tm, d_w, d_pw, d_pb, d_out)
    nc.compile()
    return nc


def _body(tc, n_enc, n_dec, d_xenc, d_tm, d_w, d_pw, d_pb, d_out):
    nc = tc.nc
    consts = tc.alloc_tile_pool(name="consts", bufs=1)
    work = tc.alloc_tile_pool(name="work", bufs=1)
    gpool = tc.alloc_tile_pool(name="gpool", bufs=2)
    ag_pool = tc.alloc_tile_pool(name="agp", bufs=10)
    ac_pool = tc.alloc_tile_pool(name="acp", bufs=6)
    ps1 = tc.alloc_tile_pool(name="ps1", bufs=3, space="PSUM")
    ps2 = tc.alloc_tile_pool(name="ps2", bufs=3, space="PSUM")

    # ---- resident constants ----
    # tm layout: [128(p), m, kpair, j, 512]; node index = (kpair*2+j)*128 + p
    tm_sb = {}
    for s2 in set((S2_GATE, S2_CAND)):
        dt_ = FP8 if s2 == "fp8" else BF16
        t = consts.tile([128, NM, 2, 2, 512], dt_, name=f"tm_sb_{s2}")
        tm_sb[s2] = t
        for m in range(NM):
            for k in range(KCH):
                row = (m * KCH + k) * 128
                nc.sync.dma_start(
                    out=t[:, m, k // 2, k % 2, :], in_=d_tm[s2][row : row + 128, :]
                )

    w_sb = {}
    for key, dt_ in d_w.items():
        shape = list(dt_.shape)
        sb_dt = BF16 if key[1] == "w" else F32
        w_sb[key] = consts.tile(shape, sb_dt, name=f"sb_{key}")
        nc.sync.dma_start(out=w_sb[key][:, :], in_=dt_[:, :])
    pw_sb = consts.tile([128, 2], BF16, name="pw_sb")
    nc.sync.dma_start(out=pw_sb[64:128, :], in_=d_pw[:, :])
    pb_sb = consts.tile([1, 1], F32, name="pb_sb")
    nc.sync.dma_start(out=pb_sb, in_=d_pb[:, :])

    # ---- persistent state ----
    X0 = work.tile([C0, BI], BF16, name="X0")  # [h0 ; x]
    X0c = work.tile([C0, BI], BF16, name="X0c")  # [r0*h0 ; x]
    X1 = work.tile([C1, BI], BF16, name="X1")  # [h0 ; h1]
    X1c = work.tile([C1, BI], BF16, name="X1c")  # [h0 ; r1*h1]
    outrow = work.tile([1, BI], F32, name="outrow")

    nc.gpsimd.memset(X0[0:U, :], 0.0)
    nc.gpsimd.memset(X0c[0:U, :], 0.0)
    nc.gpsimd.memset(X1[:, :], 0.0)
    nc.gpsimd.memset(X1c[:, :], 0.0)

    tc.strict_bb_all_engine_barrier()

    def cell_phases(lyr, X, Xc, c_in, wg, bg, wc, bc, h_src, r_dst, h_dst, post):
        """One DCGRU cell, split into per-batch-pair phases.

        Layer-l elementwise state lives at partition base l*64 so every
        two-tensor DVE op has matching input bases (h1 sits at rows 64:128 of
        X1).  The gate output layout is [r; u] for layer 0 and [u; r] for
        layer 1 (weights pre-flipped on host), so r shares a base with h; the
        u half is moved across with one single-src copy per pair.
        """
        s2g, s2c = (S2_GATE, S2_CAND)
        gdt = FP8 if s2g == "fp8" else BF16
        cdt = FP8 if s2c == "fp8" else BF16
        sl = slice(lyr * U, (lyr + 1) * U)  # this layer's partition rows
        u_src = slice(U, 2 * U) if lyr == 0 else slice(0, U)  # u half of RU
        r_src = slice(0, U) if lyr == 0 else slice(U, 2 * U)  # r half of RU
        RU = gpool.tile([2 * U, BI], BF16, tag="RU", name="RU", bufs=2)
        Uu = gpool.tile([2 * U, BI], BF16, tag="Uu", name="Uu", bufs=2)
        Wu = gpool.tile([2 * U, BI], BF16, tag="Wu", name="Wu", bufs=2)
        uh = gpool.tile([2 * U, BI], BF16, tag="uh", name="uh", bufs=2)
        Ct = gpool.tile([2 * U, BI], BF16, tag="Ct", name="Ct", bufs=2)
        wct = gpool.tile([2 * U, BI], BF16, tag="wct", name="wct", bufs=2)
        ag = {}
        ac = {}

        def gate_phase(p):
            # stage 1: A_m = X @ Wg_m for m=1..4 (m=0 folded into stage 2)
            for b in (2 * p, 2 * p + 1):
                for k in range(KCH):
                    pg = ps1.tile([128, 512], F32, tag="s1g", name="pg")
                    lhsT = X[0:c_in, b * N + k * 128 : b * N + (k + 1) * 128]
                    nc.tensor.matmul(
                        pg, lhsT, wg[:, 128:640], start=True, stop=True
                    )
                    if k % 2 == 0:
                        a = ag_pool.tile(
                            [128, 2, NM - 1, 128], gdt, tag="ag", name="ag"
                        )
                        ag[(b, k // 2)] = a
                    nc.scalar.copy(out=ag[(b, k // 2)][:, k % 2, :, :], in_=pg)
            # stage 2: acc = X @ Wg_0 + sum_{m>0} (T_m A_m)^T, fused sigmoid
            for b in (2 * p, 2 * p + 1):
                acc = ps2.tile([128, 512], F32, tag="s2", name="accg")
                nc.tensor.matmul(
                    acc,
                    wg[:, 0:128],
                    X[0:c_in, b * N : (b + 1) * N],
                    start=True,
                    stop=False,
                )
                if s2g == "fp8":
                    for m in range(1, NM):
                        for kp in range(2):
                            nc.tensor.matmul(
                                acc,
                                ag[(b, kp)][:, :, m - 1, :],
                                tm_sb[s2g][:, m, kp],
                                start=False,
                                stop=(m == NM - 1 and kp == 1),
                                perf_mode=DR,
                            )
                else:
                    for m in range(1, NM):
                        for k in range(KCH):
                            nc.tensor.matmul(
                                acc,
                                ag[(b, k // 2)][:, k % 2, m - 1, :],
                                tm_sb[s2g][:, m, k // 2, k % 2, :],
                                start=False,
                                stop=(m == NM - 1 and k == KCH - 1),
                            )
                bcols = slice(b * N, (b + 1) * N)
                nc.scalar.activation(
                    out=RU[:, bcols], in_=acc, func=AF.Sigmoid,
                    bias=bg[:, 0:1], scale=1.0,
                )
            pcols = slice(2 * p * N, 2 * (p + 1) * N)
            # move u to this layer's partition rows (single-src cross-base copy)
            nc.vector.tensor_copy(out=Uu[sl, pcols], in_=RU[u_src, pcols])
            # r*h -> candidate input rows; u*h and (1-u) for the GRU blend
            nc.vector.tensor_mul(
                out=r_dst(pcols), in0=RU[r_src, pcols], in1=h_src(pcols)
            )
            nc.vector.tensor_mul(
                out=uh[sl, pcols], in0=Uu[sl, pcols], in1=h_src(pcols)
            )
            nc.vector.tensor_scalar(
                out=Wu[sl, pcols], in0=Uu[sl, pcols],
                scalar1=-1.0, scalar2=1.0,
                op0=mybir.AluOpType.mult, op1=mybir.AluOpType.add,
            )

        def cand_phase(p):
            # stage 1: m=1..4 only; two chunks share one PSUM bank (256 cols each)
            for b in (2 * p, 2 * p + 1):
                for kp in range(2):
                    pc = ps1.tile([128, 512], F32, tag="s1c", name="pc", bufs=2)
                    for j in range(2):
                        k = kp * 2 + j
                        lhsT = Xc[0:c_in, b * N + k * 128 : b * N + (k + 1) * 128]
                        nc.tensor.matmul(
                            pc[:, j * 256 : (j + 1) * 256],
                            lhsT,
                            wc[:, U : NM * U],
                            start=True,
                            stop=True,
                        )
                    if b % 2 == 0:
                        a = ac_pool.tile(
                            [128, 2, NM - 1, 2, U], cdt, tag="ac", name="ac"
                        )
                        ac[(p, kp)] = a
                    src = pc[:, :].rearrange("p (j m u) -> p j m u", j=2, m=NM - 1)
                    nc.vector.tensor_copy(
                        out=ac[(p, kp)][:, :, :, b % 2, :], in_=src
                    )
            # stage 2: identity fold (col-tiled pair) + diffusion, fused tanh
            acc = ps2.tile([128, 512], F32, tag="s2", name="accc")
            for half in range(2):
                b = 2 * p + half
                nc.tensor.matmul(
                    acc[half * U : (half + 1) * U, :],
                    wc[:, 0:U],
                    Xc[0:c_in, b * N : (b + 1) * N],
                    start=True,  # per-partition zero region: each half starts its own rows
                    stop=False,
                    tile_position=(0, half * U),
                    # sim's group tracker isn't partition-base-aware; half 1 would
                    # falsely collide with half 0's pending group
                    skip_group_check=True,
                )
            if s2c == "fp8":
                for m in range(1, NM):
                    for kp in range(2):
                        nc.tensor.matmul(
                            acc,
                            ac[(p, kp)][:, :, m - 1, :, :],
                            tm_sb[s2c][:, m, kp],
                            start=False,
                            stop=(m == NM - 1 and kp == 1),
                            perf_mode=DR,
                            skip_group_check=True,
                        )
            else:
                for m in range(1, NM):
                    for k in range(KCH):
                        nc.tensor.matmul(
                            acc,
                            ac[(p, k // 2)][:, k % 2, m - 1, :, :],
                            tm_sb[s2c][:, m, k // 2, k % 2, :],
                            start=False,
                            stop=(m == NM - 1 and k == KCH - 1),
                            skip_group_check=True,
                        )
            for half in range(2):
                b = 2 * p + half
                bcols = slice(b * N, (b + 1) * N)
                nc.scalar.activation(
                    out=Ct[sl, bcols],
                    in_=acc[half * U : (half + 1) * U, :],
                    func=AF.Tanh, bias=bc[:, 0:1], scale=1.0,
                )
                # h_new = u*h + (1-u)*c
                nc.vector.tensor_mul(
                    out=wct[sl, bcols], in0=Wu[sl, bcols], in1=Ct[sl, bcols]
                )
                nc.vector.tensor_add(
                    out=h_dst(bcols), in0=uh[sl, bcols], in1=wct[sl, bcols]
                )
            post(p)

        return gate_phase, cand_phase

    def l0_h_src(cols):
        return X0[0:U, cols]

    def l0_r_dst(cols):
        return X0c[0:U, cols]

    def l0_h_dst(cols):
        return X0[0:U, cols]

    def l0_post(p):
        pcols = slice(2 * p * N, 2 * (p + 1) * N)
        nc.vector.tensor_copy(out=X1[0:U, pcols], in_=X0[0:U, pcols])
        nc.vector.tensor_copy(out=X1c[0:U, pcols], in_=X0[0:U, pcols])

    def l1_h_src(cols):
        return X1[U:C1, cols]

    def l1_r_dst(cols):
        return X1c[U:C1, cols]

    def l1_h_dst(cols):
        return X1[U:C1, cols]

    def l1_post(p):
        pass

    def proj_phase(p):
        # projection for pair p: out = h1 . pw + pb (row 0 of pp)
        for q in (2 * p, 2 * p + 1):
            pp = ps2.tile([128, 512], F32, tag="s2", name="pp")
            nc.tensor.matmul(
                pp[0:2, :],
                pw_sb[64:128, :],
                X1[U:C1, q * 512 : (q + 1) * 512],
                start=True,
                stop=True,
            )
            nc.scalar.activation(
                out=outrow[0:1, q * 512 : (q + 1) * 512],
                in_=pp[0:1, :],
                func=AF.Identity,
                bias=pb_sb[:, 0:1],
                scale=1.0,
            )

    def run_step(pfx, dec_t=None):
        g0, c0 = cell_phases(
            0, X0, X0c, C0,
            w_sb[f"{pfx}wg0"], w_sb[f"{pfx}bg0"], w_sb[f"{pfx}wc0"],
            w_sb[f"{pfx}bc0"], l0_h_src, l0_r_dst, l0_h_dst, l0_post,
        )
        g1, c1 = cell_phases(
            1, X1, X1c, C1,
            w_sb[f"{pfx}wg1"], w_sb[f"{pfx}bg1"], w_sb[f"{pfx}wc1"],
            w_sb[f"{pfx}bc1"], l1_h_src, l1_r_dst, l1_h_dst, l1_post,
        )
        g0(0); g0(1); c0(0); c0(1)
        g1(0); g1(1); c1(0)
        if dec_t is None:
            c1(1)
        else:
            proj_phase(0)
            c1(1)
            proj_phase(1)
            nc.vector.tensor_copy(out=X0[U:C0, :], in_=outrow)
            nc.vector.tensor_copy(out=X0c[U:C0, :], in_=outrow)
            nc.sync.dma_start(out=d_out[dec_t : dec_t + 1, :], in_=outrow)

    # ================= encoder =================
    for t in range(n_enc):
        nc.sync.dma_start(out=X0[U:C0, :], in_=d_xenc[t : t + 1, :])
        nc.sync.dma_start(out=X0c[U:C0, :], in_=d_xenc[t : t + 1, :])
        run_step("e")

    # ================= decoder =================
    nc.vector.memset(X0[U:C0, :], 0.0)
    nc.vector.memset(X0c[U:C0, :], 0.0)
    for t in range(n_dec):
        run_step("d", dec_t=t)

    for pool in (ps2, ps1, ac_pool, ag_pool, gpool, work, consts):
        pool.release()


# --------------------------------------------------------------------------
# host-side packing
# --------------------------------------------------------------------------
def _prep_shared(inputs):
    bf = mybir.dt.np(BF16)
    f8 = mybir.dt.np(FP8)
    sup = np.asarray(inputs["supports"], np.float64)
    eye = np.eye(N, dtype=np.float64)
    tms = [
        eye,
        sup[0],
        2.0 * (sup[0] @ sup[0]) - eye,
        sup[1],
        2.0 * (sup[1] @ sup[1]) - eye,
    ]
    tmats = np.stack([t.T for t in tms]).astype(np.float32)  # [m, j, i]
    tmats = tmats.reshape(NM * KCH * 128, 512)

    shared = {}
    for s2 in set((S2_GATE, S2_CAND)):
        dt_ = f8 if s2 == "fp8" else bf
        shared[f"tm_{s2}"] = np.ascontiguousarray(tmats.astype(dt_))
    for pfx, name in (("e", "enc"), ("d", "dec")):
        for lyr, c_in in ((0, C0), (1, C1)):
            wg = np.asarray(inputs[f"{name}{lyr}_Wg"], np.float32).reshape(
                c_in, NM * 2 * U
            )
            wc = np.asarray(inputs[f"{name}{lyr}_Wc"], np.float32).reshape(
                c_in, NM * U
            )
            bg = np.asarray(inputs[f"{name}{lyr}_bg"], np.float32)
            bc = np.asarray(inputs[f"{name}{lyr}_bc"], np.float32)
            if lyr == 0:
                perm = np.r_[1:c_in, 0]  # rows [h..., x]
                wg = wg[perm]
                wc = wc[perm]
            else:
                # layer-1 gate layout is [u; r] (see cell_phases): swap the
                # r/u column halves inside each m block, and the bias halves
                wg = np.ascontiguousarray(
                    wg.reshape(c_in, NM, 2, U)[:, :, ::-1, :].reshape(c_in, NM * 2 * U)
                )
                bg = np.concatenate([bg[U:], bg[:U]])
            shared[f"{pfx}wg{lyr}"] = np.ascontiguousarray(wg.astype(bf))
            shared[f"{pfx}wc{lyr}"] = np.ascontiguousarray(wc.astype(bf))
            shared[f"{pfx}bg{lyr}"] = np.ascontiguousarray(bg.reshape(2 * U, 1))
            shared[f"{pfx}bc{lyr}"] = np.ascontiguousarray(bc.reshape(U, 1))
    pw = np.asarray(inputs["proj_W"], np.float32).reshape(U, 1)
    shared["pw"] = np.ascontiguousarray(
        np.concatenate([pw, np.zeros((U, 1), np.float32)], axis=1).astype(bf)
    )
    shared["pb"] = np.asarray(inputs["proj_b"], np.float32).reshape(1, 1)
    return shared


def _make_in_maps(inputs, n_enc=T_ENC):
    bf = mybir.dt.np(BF16)
    shared = _prep_shared(inputs)
    x = np.asarray(inputs["inputs"], np.float32)  # (T, B, N)
    in_maps = []
    for c in range(NCORES):
        m = dict(shared)
        m["xenc"] = np.ascontiguousarray(
            x[:n_enc, c * BL : (c + 1) * BL, :].reshape(n_enc, BI).astype(bf)
        )
        in_maps.append(m)
    return in_maps


_PROG_CACHE = {}


def _get_program(n_enc=T_ENC, n_dec=HOR):
    key = (n_enc, n_dec)
    if key not in _PROG_CACHE:
        _PROG_CACHE[key] = _build_program(n_enc, n_dec)
    return _PROG_CACHE[key]


def _run(inputs, n_enc=T_ENC, n_dec=HOR, **kw):
    nc = _get_program(n_enc, n_dec)
    in_maps = _make_in_maps(inputs, n_enc)
    res = bass_utils.run_bass_kernel_spmd(nc, in_maps, core_ids=list(range(NCORES)), **kw)
    out = np.empty((n_dec, B, N), np.float32)
    for c in range(NCORES):
        out[:, c * BL : (c + 1) * BL, :] = res.results[c]["outs"].reshape(n_dec, BL, N)
    return out.reshape(n_dec, B, N), res


def kernel(**inputs) -> np.ndarray:
    out, _ = _run(inputs)
    return out.reshape(HOR, B, N)


# revision 10
# speedup vs baseline: 1.4886x; 1.0671x over previous
"""DCRNN (2-layer encoder/decoder DCGRU, N=512 nodes, B=32, U=64, K=2, 2 supports)
Trainium2 Bass/Tile kernel, data-parallel over batch across 8 NeuronCores.

Formulation: gconv(X) = sum_m T_m @ X @ W_m with T_m precomputed on host
(m=0 is the identity and is folded into stage 2 as a direct X @ W_0 matmul).
  stage 1 (dense):     A_m = X @ W_m, m=1..4  (X-chunk as lhsT -> node-major A)
  stage 2 (diffusion): out = X @ W_0 + sum_m (T_m A_m)^T, accumulated in PSUM,
                       bias+sigmoid/tanh fused into the PSUM->SBUF activation.
All matmul operands bf16 (or fp8e4m3 with DoubleRow for the diffusion stage:
two 128-row node chunks contracted per matmul at 2x rate). State tiles bf16.
Layout avoids every partition-shift copy:
  X0  [65,BI]  rows 0:64 h0,    row 64 x      (L0 gate lhsT)
  X0c [65,BI]  rows 0:64 r0*h0, row 64 x      (L0 cand lhsT)
  X1  [128,BI] rows 0:64 h0',   rows 64:128 h1        (L1 gate lhsT)
  X1c [128,BI] rows 0:64 h0',   rows 64:128 r1*h1     (L1 cand lhsT)
h1 and r1*h1 are written at partition base 64 directly by DVE (cross-base ok).
"""

import sys

sys.path.insert(0, "/opt/trn_rl_repo")

import numpy as np

import concourse.bass as bass
import concourse.mybir as mybir
import concourse.tile as tile
from concourse import bacc, bass_utils

# Model dims (fixed by the problem)
N = 512
T_ENC = 12
HOR = 12
U = 64
NM = 5  # diffusion matrices (I + 2 per support * 2 supports)
B = 32
NCORES = 8
BL = B // NCORES  # local batch = 4
BI = BL * N  # 2048: the (b, node) free dim
C0 = 1 + U  # 65 input channels, layer 0
C1 = U + U  # 128 input channels, layer 1
KCH = N // 128  # 4 node chunks

F32 = mybir.dt.float32
BF16 = mybir.dt.bfloat16
FP8 = mybir.dt.float8e4
AF = mybir.ActivationFunctionType
DR = mybir.MatmulPerfMode.DoubleRow

# stage-2 diffusion dtype per path: "fp8" (DoubleRow) or "bf16"
import os as _os

S2_GATE = _os.environ.get("S2_GATE", "fp8")
S2_CAND = _os.environ.get("S2_CAND", "fp8")
# walrus ldw-opt pass chokes on DoubleRow Ldweights; only enable when no fp8 path
LDW_OPT = _os.environ.get("LDW_OPT", "0") == "1"

_ldw_patched = False


def _patch_ldw_opt():
    global _ldw_patched
    if _ldw_patched or not LDW_OPT:
        return
    _ldw_patched = True
    orig = bass_utils.bir_verify_and_optimise

    def patched(tmpdir, inp="bir.json", outp="file.neff", arch=None, *, dve_root=None):
        import concourse.bass_utils as bu

        real_run = bu.run_command

        def run_hook(cmd, **kw):
            cmd = [
                c.replace("--enable-ldw-opt=false", "--enable-ldw-opt=true")
                for c in cmd
            ]
            return real_run(cmd, **kw)

        bu.run_command = run_hook
        try:
            return orig(tmpdir, inp, outp, arch, dve_root=dve_root)
        finally:
            bu.run_command = real_run

    bass_utils.bir_verify_and_optimise = patched


def _build_program(n_enc=T_ENC, n_dec=HOR):
    _patch_ldw_opt()
    nc = bacc.Bacc("TRN2", target_bir_lowering=False, debug=False)

    # ---- DRAM I/O ----
    d_xenc = nc.dram_tensor("xenc", [n_enc, BI], BF16, kind="ExternalInput")
    d_tm = {}
    dts = set((S2_GATE, S2_CAND))
    for s2 in dts:
        dt_ = FP8 if s2 == "fp8" else BF16
        d_tm[s2] = nc.dram_tensor(
            f"tm_{s2}", [NM * KCH * 128, 512], dt_, kind="ExternalInput"
        )
    d_w = {}
    for pfx in ("e", "d"):
        for lyr, c_in in ((0, C0), (1, C1)):
            d_w[f"{pfx}wg{lyr}"] = nc.dram_tensor(
                f"{pfx}wg{lyr}", [c_in, NM * 2 * U], BF16, kind="ExternalInput"
            )
            d_w[f"{pfx}wc{lyr}"] = nc.dram_tensor(
                f"{pfx}wc{lyr}", [c_in, NM * U], BF16, kind="ExternalInput"
            )
            d_w[f"{pfx}bg{lyr}"] = nc.dram_tensor(
                f"{pfx}bg{lyr}", [2 * U, 1], F32, kind="ExternalInput"
            )
            d_w[f"{pfx}bc{lyr}"] = nc.dram_tensor(
                f"{pfx}bc{lyr}", [U, 1], F32, kind="ExternalInput"
            )
    d_pw = nc.dram_tensor("pw", [U, 2], BF16, kind="ExternalInput")
    d_pb = nc.dram_tensor("pb", [1, 1], F32, kind="ExternalInput")
    d_out = nc.dram_tensor("outs", [n_dec, BI], F32, kind="ExternalOutput")

    with tile.TileContext(nc) as tc:
        _body(tc, n_enc, n_dec, d_xenc, d_tm, d_w, d_pw, d_pb, d_out)
    nc.compile()
    return nc


def _body(tc, n_enc, n_dec, d_xenc, d_tm, d_w, d_pw, d_pb, d_out):
    nc = tc.nc
    consts = tc.alloc_tile_pool(name="consts", bufs=1)
    work = tc.alloc_tile_pool(name="work", bufs=1)
    gpool = tc.alloc_tile_pool(name="gpool", bufs=2)
    ag_pool = tc.alloc_tile_pool(name="agp", bufs=10)
    ac_pool = tc.alloc_tile_pool(name="acp", bufs=6)
    ps1 = tc.alloc_tile_pool(name="ps1", bufs=2, space="PSUM")
    ps2 = tc.alloc_tile_pool(name="ps2", bufs=2, space="PSUM")

    # ---- resident constants ----
    # tm layout: [128(p), m, kpair, j, 512]; node index = (kpair*2+j)*128 + p
    tm_sb = {}
    for s2 in set((S2_GATE, S2_CAND)):
        dt_ = FP8 if s2 == "fp8" else BF16
        t = consts.tile([128, NM, 2, 2, 512], dt_, name=f"tm_sb_{s2}")
        tm_sb[s2] = t
        for m in range(NM):
            for k in range(KCH):
                row = (m * KCH + k) * 128
                nc.sync.dma_start(
                    out=t[:, m, k // 2, k % 2, :], in_=d_tm[s2][row : row + 128, :]
                )

    w_sb = {}
    for key, dt_ in d_w.items():
        shape = list(dt_.shape)
        sb_dt = BF16 if key[1] == "w" else F32
        w_sb[key] = consts.tile(shape, sb_dt, name=f"sb_{key}")
        nc.sync.dma_start(out=w_sb[key][:, :], in_=dt_[:, :])
    pw_sb = consts.tile([128, 2], BF16, name="pw_sb")
    nc.sync.dma_start(out=pw_sb[64:128, :], in_=d_pw[:, :])
    pb_sb = consts.tile([1, 1], F32, name="pb_sb")
    nc.sync.dma_start(out=pb_sb, in_=d_pb[:, :])

    # ---- persistent state ----
    X0 = work.tile([C0, BI], BF16, name="X0")  # [h0 ; x]
    X0c = work.tile([C0, BI], BF16, name="X0c")  # [r0*h0 ; x]
    X1 = work.tile([C1, BI], BF16, name="X1")  # [h0 ; h1]
    X1c = work.tile([C1, BI], BF16, name="X1c")  # [h0 ; r1*h1]
    outrow = work.tile([1, BI], F32, name="outrow")

    nc.gpsimd.memset(X0[0:U, :], 0.0)
    nc.gpsimd.memset(X0c[0:U, :], 0.0)
    nc.gpsimd.memset(X1[:, :], 0.0)
    nc.gpsimd.memset(X1c[:, :], 0.0)

    tc.strict_bb_all_engine_barrier()

    def cell_phases(lyr, X, Xc, c_in, wg, bg, wc, bc, h_src, r_dst, h_dst, post):
        """One DCGRU cell, split into per-batch-pair phases.

        Layer-l elementwise state lives at partition base l*64 so every
        two-tensor DVE op has matching input bases (h1 sits at rows 64:128 of
        X1).  The gate output layout is [r; u] for layer 0 and [u; r] for
        layer 1 (weights pre-flipped on host), so r shares a base with h; the
        u half is moved across with one single-src copy per pair.
        """
        s2g, s2c = (S2_GATE, S2_CAND)
        gdt = FP8 if s2g == "fp8" else BF16
        cdt = FP8 if s2c == "fp8" else BF16
        sl = slice(lyr * U, (lyr + 1) * U)  # this layer's partition rows
        u_src = slice(U, 2 * U) if lyr == 0 else slice(0, U)  # u half of RU
        r_src = slice(0, U) if lyr == 0 else slice(U, 2 * U)  # r half of RU
        RU = gpool.tile([2 * U, BI], BF16, tag="RU", name="RU", bufs=2)
        Uu = gpool.tile([2 * U, BI], BF16, tag="Uu", name="Uu", bufs=2)
        Wu = gpool.tile([2 * U, BI], BF16, tag="Wu", name="Wu", bufs=2)
        uh = gpool.tile([2 * U, BI], BF16, tag="uh", name="uh", bufs=2)
        Ct = gpool.tile([2 * U, BI], BF16, tag="Ct", name="Ct", bufs=2)
        wct = gpool.tile([2 * U, BI], BF16, tag="wct", name="wct", bufs=2)
        ag = {}
        ac = {}

        def gate_phase(p):
            # stage 1: A_m = X @ Wg_m for m=1..4 (m=0 folded into stage 2).
            # Two node chunks share a 2-bank PSUM pair tile -> one wide copy.
            for b in (2 * p, 2 * p + 1):
                for kp in range(2):
                    pg = ps1.tile([128, 2, 512], F32, tag="s1g", name="pg")
                    for j in range(2):
                        k = kp * 2 + j
                        lhsT = X[0:c_in, b * N + k * 128 : b * N + (k + 1) * 128]
                        nc.tensor.matmul(
                            pg[:, j, :], lhsT, wg[:, 128:640], start=True, stop=True
                        )
                    a = ag_pool.tile(
                        [128, 2, NM - 1, 128], gdt, tag="ag", name="ag"
                    )
                    ag[(b, kp)] = a
                    nc.scalar.copy(out=a[:, :, :, :], in_=pg.rearrange(
                        "p j (m c) -> p j m c", m=NM - 1))
            # stage 2: acc = X @ Wg_0 + sum_{m>0} (T_m A_m)^T, fused sigmoid.
            # The two batches of the pair share a 2-bank acc -> one wide act.
            accp = ps2.tile([128, 2, 512], F32, tag="s2", name="accg")
            for half, b in enumerate((2 * p, 2 * p + 1)):
                acc = accp[:, half, :]
                nc.tensor.matmul(
                    acc,
                    wg[:, 0:128],
                    X[0:c_in, b * N : (b + 1) * N],
                    start=True,
                    stop=False,
                )
                if s2g == "fp8":
                    for m in range(1, NM):
                        for kp in range(2):
                            nc.tensor.matmul(
                                acc,
                                ag[(b, kp)][:, :, m - 1, :],
                                tm_sb[s2g][:, m, kp],
                                start=False,
                                stop=(m == NM - 1 and kp == 1),
                                perf_mode=DR,
                            )
                else:
                    for m in range(1, NM):
                        for k in range(KCH):
                            nc.tensor.matmul(
                                acc,
                                ag[(b, k // 2)][:, k % 2, m - 1, :],
                                tm_sb[s2g][:, m, k // 2, k % 2, :],
                                start=False,
                                stop=(m == NM - 1 and k == KCH - 1),
                            )
            pcols = slice(2 * p * N, 2 * (p + 1) * N)
            nc.scalar.activation(
                out=RU[:, pcols], in_=accp.rearrange("p j c -> p (j c)"),
                func=AF.Sigmoid, bias=bg[:, 0:1], scale=1.0,
            )
            # move u to this layer's partition rows (single-src cross-base copy)
            nc.vector.tensor_copy(out=Uu[sl, pcols], in_=RU[u_src, pcols])
            # r*h -> candidate input rows; u*h and (1-u) for the GRU blend
            nc.vector.tensor_mul(
                out=r_dst(pcols), in0=RU[r_src, pcols], in1=h_src(pcols)
            )
            nc.vector.tensor_mul(
                out=uh[sl, pcols], in0=Uu[sl, pcols], in1=h_src(pcols)
            )
            nc.vector.tensor_scalar(
                out=Wu[sl, pcols], in0=Uu[sl, pcols],
                scalar1=-1.0, scalar2=1.0,
                op0=mybir.AluOpType.mult, op1=mybir.AluOpType.add,
            )

        def cand_phase(p):
            # stage 1: m=1..4 only; all 4 chunks of one batch share a 2-bank
            # pair tile (256 cols each) -> one wide copy per batch
            for half, b in enumerate((2 * p, 2 * p + 1)):
                pc = ps1.tile([128, 2, 512], F32, tag="s1g", name="pc")
                for k in range(KCH):
                    lhsT = Xc[0:c_in, b * N + k * 128 : b * N + (k + 1) * 128]
                    nc.tensor.matmul(
                        pc[:, k // 2, (k % 2) * 256 : (k % 2) * 256 + 256],
                        lhsT,
                        wc[:, U : NM * U],
                        start=True,
                        stop=True,
                        skip_group_check=(k % 2 == 1),
                    )
                if half == 0:
                    a = ac_pool.tile(
                        [128, 2, 2, NM - 1, 2, U], cdt, tag="ac", name="ac"
                    )
                    ac[p] = a
                srcv = pc.rearrange("p kp (j m u) -> p kp j m u", j=2, m=NM - 1)
                nc.vector.tensor_copy(out=ac[p][:, :, :, :, half, :], in_=srcv)
            # stage 2: identity fold (col-tiled pair) + diffusion, fused tanh
            accp = ps2.tile([128, 2, 512], F32, tag="s2", name="accc")
            acc = accp[:, 0, :]
            for half in range(2):
                b = 2 * p + half
                nc.tensor.matmul(
                    acc[half * U : (half + 1) * U, :],
                    wc[:, 0:U],
                    Xc[0:c_in, b * N : (b + 1) * N],
                    start=True,  # per-partition zero region: each half starts its own rows
                    stop=False,
                    tile_position=(0, half * U),
                    # sim's group tracker isn't partition-base-aware; half 1 would
                    # falsely collide with half 0's pending group
                    skip_group_check=True,
                )
            if s2c == "fp8":
                for m in range(1, NM):
                    for kp in range(2):
                        nc.tensor.matmul(
                            acc,
                            ac[p][:, kp, :, m - 1, :, :],
                            tm_sb[s2c][:, m, kp],
                            start=False,
                            stop=(m == NM - 1 and kp == 1),
                            perf_mode=DR,
                            skip_group_check=True,
                        )
            else:
                for m in range(1, NM):
                    for k in range(KCH):
                        nc.tensor.matmul(
                            acc,
                            ac[p][:, k // 2, k % 2, m - 1, :, :],
                            tm_sb[s2c][:, m, k // 2, k % 2, :],
                            start=False,
                            stop=(m == NM - 1 and k == KCH - 1),
                            skip_group_check=True,
                        )
            for half in range(2):
                b = 2 * p + half
                bcols = slice(b * N, (b + 1) * N)
                nc.scalar.activation(
                    out=Ct[sl, bcols],
                    in_=acc[half * U : (half + 1) * U, :],
                    func=AF.Tanh, bias=bc[:, 0:1], scale=1.0,
                )
            pcols = slice(2 * p * N, 2 * (p + 1) * N)
            # h_new = u*h + (1-u)*c
            nc.vector.tensor_mul(
                out=wct[sl, pcols], in0=Wu[sl, pcols], in1=Ct[sl, pcols]
            )
            nc.vector.tensor_add(
                out=h_dst(pcols), in0=uh[sl, pcols], in1=wct[sl, pcols]
            )
            post(p)

        return gate_phase, cand_phase

    def l0_h_src(cols):
        return X0[0:U, cols]

    def l0_r_dst(cols):
        return X0c[0:U, cols]

    def l0_h_dst(cols):
        return X0[0:U, cols]

    def l0_post(p):
        pcols = slice(2 * p * N, 2 * (p + 1) * N)
        nc.vector.tensor_copy(out=X1[0:U, pcols], in_=X0[0:U, pcols])
        nc.vector.tensor_copy(out=X1c[0:U, pcols], in_=X0[0:U, pcols])

    def l1_h_src(cols):
        return X1[U:C1, cols]

    def l1_r_dst(cols):
        return X1c[U:C1, cols]

    def l1_h_dst(cols):
        return X1[U:C1, cols]

    def l1_post(p):
        pass

    def proj_phase(p):
        # projection for pair p: out = h1 . pw + pb (row 0 of pp)
        for q in (2 * p, 2 * p + 1):
            pp = ps2.tile([128, 2, 512], F32, tag="s2", name="pp")
            nc.tensor.matmul(
                pp[0:2, 0, :],
                pw_sb[64:128, :],
                X1[U:C1, q * 512 : (q + 1) * 512],
                start=True,
                stop=True,
            )
            nc.scalar.activation(
                out=outrow[0:1, q * 512 : (q + 1) * 512],
                in_=pp[0:1, 0, :],
                func=AF.Identity,
                bias=pb_sb[:, 0:1],
                scale=1.0,
            )

    def run_step(pfx, dec_t=None):
        g0, c0 = cell_phases(
            0, X0, X0c, C0,
            w_sb[f"{pfx}wg0"], w_sb[f"{pfx}bg0"], w_sb[f"{pfx}wc0"],
            w_sb[f"{pfx}bc0"], l0_h_src, l0_r_dst, l0_h_dst, l0_post,
        )
        g1, c1 = cell_phases(
            1, X1, X1c, C1,
            w_sb[f"{pfx}wg1"], w_sb[f"{pfx}bg1"], w_sb[f"{pfx}wc1"],
            w_sb[f"{pfx}bc1"], l1_h_src, l1_r_dst, l1_h_dst, l1_post,
        )
        g0(0); g0(1); c0(0); c0(1)
        g1(0); g1(1); c1(0)
        if dec_t is None:
            c1(1)
        else:
            proj_phase(0)
            c1(1)
            proj_phase(1)
            nc.vector.tensor_copy(out=X0[U:C0, :], in_=outrow)
            nc.vector.tensor_copy(out=X0c[U:C0, :], in_=outrow)
            nc.sync.dma_start(out=d_out[dec_t : dec_t + 1, :], in_=outrow)

    # ================= encoder =================
    for t in range(n_enc):
        nc.sync.dma_start(out=X0[U:C0, :], in_=d_xenc[t : t + 1, :])
        nc.sync.dma_start(out=X0c[U:C0, :], in_=d_xenc[t : t + 1, :])
        run_step("e")

    # ================= decoder =================
    nc.vector.memset(X0[U:C0, :], 0.0)
    nc.vector.memset(X0c[U:C0, :], 0.0)
    for t in range(n_dec):
        run_step("d", dec_t=t)

    for pool in (ps2, ps1, ac_pool, ag_pool, gpool, work, consts):
        pool.release()


# --------------------------------------------------------------------------
# host-side packing
# --------------------------------------------------------------------------
def _prep_shared(inputs):
    bf = mybir.dt.np(BF16)
    f8 = mybir.dt.np(FP8)
    sup = np.asarray(inputs["supports"], np.float64)
    eye = np.eye(N, dtype=np.float64)
    tms = [
        eye,
        sup[0],
        2.0 * (sup[0] @ sup[0]) - eye,
        sup[1],
        2.0 * (sup[1] @ sup[1]) - eye,
    ]
    tmats = np.stack([t.T for t in tms]).astype(np.float32)  # [m, j, i]
    tmats = tmats.reshape(NM * KCH * 128, 512)

    shared = {}
    for s2 in set((S2_GATE, S2_CAND)):
        dt_ = f8 if s2 == "fp8" else bf
        shared[f"tm_{s2}"] = np.ascontiguousarray(tmats.astype(dt_))
    for pfx, name in (("e", "enc"), ("d", "dec")):
        for lyr, c_in in ((0, C0), (1, C1)):
            wg = np.asarray(inputs[f"{name}{lyr}_Wg"], np.float32).reshape(
                c_in, NM * 2 * U
            )
            wc = np.asarray(inputs[f"{name}{lyr}_Wc"], np.float32).reshape(
                c_in, NM * U
            )
            bg = np.asarray(inputs[f"{name}{lyr}_bg"], np.float32)
            bc = np.asarray(inputs[f"{name}{lyr}_bc"], np.float32)
            if lyr == 0:
                perm = np.r_[1:c_in, 0]  # rows [h..., x]
                wg = wg[perm]
                wc = wc[perm]
            else:
                # layer-1 gate layout is [u; r] (see cell_phases): swap the
                # r/u column halves inside each m block, and the bias halves
                wg = np.ascontiguousarray(
                    wg.reshape(c_in, NM, 2, U)[:, :, ::-1, :].reshape(c_in, NM * 2 * U)
                )
                bg = np.concatenate([bg[U:], bg[:U]])
            shared[f"{pfx}wg{lyr}"] = np.ascontiguousarray(wg.astype(bf))
            shared[f"{pfx}wc{lyr}"] = np.ascontiguousarray(wc.astype(bf))
            shared[f"{pfx}bg{lyr}"] = np.ascontiguousarray(bg.reshape(2 * U, 1))
            shared[f"{pfx}bc{lyr}"] = np.ascontiguousarray(bc.reshape(U, 1))
    pw = np.asarray(inputs["proj_W"], np.float32).reshape(U, 1)
    shared["pw"] = np.ascontiguousarray(
        np.concatenate([pw, np.zeros((U, 1), np.float32)], axis=1).astype(bf)
    )
    shared["pb"] = np.asarray(inputs["proj_b"], np.float32).reshape(1, 1)
    return shared


def _make_in_maps(inputs, n_enc=T_ENC):
    bf = mybir.dt.np(BF16)
    shared = _prep_shared(inputs)
    x = np.asarray(inputs["inputs"], np.float32)  # (T, B, N)
    in_maps = []
    for c in range(NCORES):
        m = dict(shared)
        m["xenc"] = np.ascontiguousarray(
            x[:n_enc, c * BL : (c + 1) * BL, :].reshape(n_enc, BI).astype(bf)
        )
        in_maps.append(m)
    return in_maps


_PROG_CACHE = {}


def _get_program(n_enc=T_ENC, n_dec=HOR):
    key = (n_enc, n_dec)
    if key not in _PROG_CACHE:
        _PROG_CACHE[key] = _build_program(n_enc, n_dec)
    return _PROG_CACHE[key]


def _run(inputs, n_enc=T_ENC, n_dec=HOR, **kw):
    nc = _get_program(n_enc, n_dec)
    in_maps = _make_in_maps(inputs, n_enc)
    res = bass_utils.run_bass_kernel_spmd(nc, in_maps, core_ids=list(range(NCORES)), **kw)
    out = np.empty((n_dec, B, N), np.float32)
    for c in range(NCORES):
        out[:, c * BL : (c + 1) * BL, :] = res.results[c]["outs"].reshape(n_dec, BL, N)
    return out.reshape(n_dec, B, N), res


def kernel(**inputs) -> np.ndarray:
    out, _ = _run(inputs)
    return out.reshape(HOR, B, N)
